# revision 20
# baseline (speedup 1.0000x reference)
"""BitLinear (input-RMSNorm + ternary-quantized linear) on 8 TRN2 NeuronCores.

Math (reference):
  xn    = x * rsqrt(mean(x^2, -1) + eps) * g
  w     = weight * rsqrt(mean(weight^2, 1) + eps)          (row RMS norm)
  am    = mean(|w|, 1)
  w_q   = sign(w) * (|w| > 0.5*am)                          (ternary)
  out   = xn @ (w_q * am * row_scale).T + bias

Kernel strategy (per core, data-parallel over B*S rows; weight replicated):
  - fp8e4 DoubleRowSwInterleave matmuls (K=256/issue, measured ~94 ns per
    N=512 MM = ~2x the bf16 MAC rate).  The ternary weight {-1,0,1} is
    EXACT in fp8; x rides as fp8(main) + fp8(residual = x - fp8(x)), two
    accumulating GEMM chains into one PSUM bank, so the matmul path loses
    nothing vs bf16 accuracy (~2e-3 rel err vs the 2e-2 gate).
  - alpha = am*rw*row_scale stays f32: broadcast across partitions via a
    tiny PE ones-outer-product per weight tile, applied in the epilogue
    STT (psum*sclx)*alpha on DVE; bias added on gpsimd in bf16.
  - x row rsqrt (sclx) commutes with the matmul (applied per-partition in
    the epilogue STT); g is per-partition in the transposed domain and
    rides the PSUM->SBUF fp8 conversion copies (skipped when g == 1).
  - |w| > 0.5*mean|w| evaluated in the raw-weight domain (rsqrt cancels).
  - x transposed on PE in f32r (exact); main fp8 = ACT copy-cast from
    PSUM, residual = DVE (psum - main) -> fp8, written straight into the
    k-pair-interleaved layouts the DR matmuls consume.
  - Software-pipelined skew: 8 weight tiles prep ahead, 2 per step after;
    (s-tile, chunk) units gate only on the 4 preps their columns need.
  - DMA: x+w loads on the sync HWDGE ring; merged bf16 stores per s-tile
    issue from the gpsimd SWDGE queue.
"""

import sys

try:
    import concourse.bass  # noqa: F401
except ImportError:
    for _p in ("/opt/trn_rl_repo", "/root/.axon_site/_ro/trn_rl_repo"):
        if _p not in sys.path:
            sys.path.insert(0, _p)

from contextlib import ExitStack

import numpy as np

import concourse.bass as bass
import concourse.mybir as mybir
import concourse.tile as tile
from concourse import bacc, bass_utils
from concourse.masks import make_identity

B, S, DIN, DOUT = 4, 4096, 2048, 2048
NCORES = 8
SC = B * S // NCORES      # 2048 rows of x per core
P = 128
KT = DIN // P             # 16 k-tiles
PAIRS = KT // 2           # 8 k-pairs per DR chain
ST = SC // P              # 16 s-tiles per core
CH = 512                  # psum chunk (one bank of fp32)
NCH = DOUT // CH          # 4 chunks
EPS = 1e-8
EHEAD = 8                 # weight tiles prepped before the main loop
RPACE = 2                 # weight tiles prepped per early main-loop step

f32 = mybir.dt.float32
f32r = mybir.dt.float32r
bf16 = mybir.dt.bfloat16
f8 = mybir.dt.float8e4
AF = mybir.ActivationFunctionType
OP = mybir.AluOpType
AX = mybir.AxisListType
SW = mybir.MatmulPerfMode.DoubleRowSwInterleave


def _skew_schedule(ehead=None, rpace=None):
    """Greedy (tile, chunk) unit order: chunk c is eligible once its 4
    preps are done; units process oldest-tile-first, <=4 per step."""
    ehead = EHEAD if ehead is None else ehead
    rpace = RPACE if rpace is None else rpace
    steps = []
    pend = []
    npreps = ehead
    arrived = 0
    for s in range(ST + 4):
        while arrived < ST and arrived <= s + 1:
            pend += [(arrived, c) for c in range(NCH)]
            arrived += 1
        elig = sorted(u for u in pend if NCH * (u[1] + 1) <= npreps and u[0] <= s)
        take = elig[:NCH]
        for u in take:
            pend.remove(u)
        steps.append(take)
        npreps = min(KT, npreps + rpace)
    assert not pend, pend
    return steps


def _fr_lifetimes(steps):
    first_use, last_use = {}, {}
    for s, us in enumerate(steps):
        for t, _ in us:
            first_use.setdefault(t, s)
            last_use[t] = s
    alive = max(
        sum(1 for t in first_use if first_use[t] <= s <= last_use[t])
        for s in range(len(steps))
    )
    return first_use, last_use, alive


def build_module(reps=1, g_one=True, ehead=None, rpace=None, ablate=()):
    ab = set(ablate.split(",")) if isinstance(ablate, str) else set(ablate)
    nc = bacc.Bacc("TRN2", target_bir_lowering=False)
    x_d = nc.declare_dram_parameter("x", [SC, DIN], f32, isOutput=False)
    w_d = nc.declare_dram_parameter("weight", [DOUT, DIN], f32, isOutput=False)
    rs_d = nc.declare_dram_parameter("row_scale", [DOUT, 1], f32, isOutput=False)
    b_d = nc.declare_dram_parameter("bias", [DOUT], f32, isOutput=False)
    g_d = nc.declare_dram_parameter("g", [DIN], f32, isOutput=False)
    o_d = nc.declare_dram_parameter("out", [SC, DOUT], bf16, isOutput=True)

    with tile.TileContext(nc) as tc, ExitStack() as ctx:
        const = ctx.enter_context(tc.tile_pool(name="const", bufs=1))
        xtp = ctx.enter_context(tc.tile_pool(name="xtp", bufs=3))
        wtp = ctx.enter_context(tc.tile_pool(name="wtp", bufs=2))
        abp = ctx.enter_context(tc.tile_pool(name="abp", bufs=2))
        epp = ctx.enter_context(tc.tile_pool(name="epp", bufs=2))
        ehead_v = EHEAD if ehead is None else ehead
        rpace_v = RPACE if rpace is None else rpace
        steps = _skew_schedule(8, 1)
        first_use, last_use, alive = _fr_lifetimes(steps)
        hip = ctx.enter_context(tc.tile_pool(name="hip", bufs=alive + 3))
        rip = ctx.enter_context(tc.tile_pool(name="rip", bufs=alive + 3))
        outp = ctx.enter_context(tc.tile_pool(name="outp", bufs=3))
        etp = ctx.enter_context(tc.tile_pool(name="etp", bufs=4))
        smp = ctx.enter_context(tc.tile_pool(name="smp", bufs=4))
        pmm = ctx.enter_context(tc.tile_pool(name="pmm", bufs=3, space="PSUM"))
        ptp = ctx.enter_context(tc.tile_pool(name="ptp", bufs=2, space="PSUM"))
        dram = ctx.enter_context(tc.tile_pool(name="dram", bufs=2, space="DRAM"))

        # ---- constants ----
        w2a = const.tile([P, PAIRS, 2, 2 * CH], f8)  # ternary, chunks 0-1 (local)
        w2b = const.tile([P, PAIRS, 2, 2 * CH], f8)  # chunks 2-3 (AllGather)
        biasb = const.tile([P, DOUT], f32)         # bias broadcast to all partitions
        biasb_bf = const.tile([P, DOUT], bf16)
        alphab = const.tile([P, DOUT], f32)        # alpha broadcast to all partitions
        ident32 = const.tile([P, P], f32)
        identbf = const.tile([P, P], bf16)
        make_identity(nc, ident32)
        nc.vector.tensor_copy(identbf, ident32)
        ident32r = ident32.bitcast(f32r)
        eps_t = const.tile([P, 1], f32)
        nc.vector.memset(eps_t, EPS)
        ones1 = const.tile([1, P], f32)
        nc.vector.memset(ones1, 1.0)
        g_col = const.tile([P, KT], f32)           # g[k], k = j*128+p -> [p, j]
        nc.gpsimd.dma_start(out=g_col, in_=g_d.rearrange("(j p) -> p j", p=P))

        # per-w-tile stats: cols 0-1 = AG shard, cols 2-9 = local chunks 0-1
        sabs = const.tile([P, 10], f32)
        rw = const.tile([P, 10], f32)
        traw = const.tile([P, 10], f32)
        ntraw = const.tile([P, 10], f32)
        alpha_sh = const.tile([P, 2], f32)
        alpha_c = const.tile([P, KT], f32)
        w2sh = const.tile([P, PAIRS, 2, 2 * P], f8)
        rs_sh = const.tile([P, 10], f32)

        # bias broadcast: DRAM [DOUT] replicated over 128 partitions
        bias_ap = b_d[:]
        nc.gpsimd.dma_start(
            out=biasb,
            in_=bass.AP(
                tensor=bias_ap.tensor, offset=bias_ap.offset,
                ap=[[0, P]] + list(bias_ap.ap),
            ),
        )
        nc.vector.tensor_copy(biasb_bf, biasb)

        # ---- weight prep ----
        # Shard: core c owns o rows [256c, 256c+256) -> ternary fp8 shard,
        # AllGathered for chunks 1-3 (w2b).  Chunk 0 (o < 512) is ALSO
        # prepped locally on every core (w2a) so the GEMM starts while the
        # collective is in flight.  AG readback rides the gpsimd queue so
        # the sync ring stays free for x loads.
        pid = nc.sync.partition_id()

        def prep_core(w_src_ap, rs_src_ap, col, e_sink):
            """stats + ternarize one o-tile; e_sink(e_t) consumes the ternary."""
            w_t = wtp.tile([P, DIN], f32, name="wt")
            nc.sync.dma_start(out=w_t, in_=w_src_ap)
            nc.sync.dma_start(out=rs_sh[:, col : col + 1], in_=rs_src_ap)
            scr4 = smp.tile([P, 4], f32, name="scr4")
            for c in range(4):
                dump = pmm.tile([P, CH], f32, name="dump", bufs=1)
                nc.scalar.activation(
                    dump, w_t[:, c * CH : (c + 1) * CH], AF.Square,
                    accum_out=scr4[:, c : c + 1],
                )
            nc.vector.tensor_tensor(scr4[:, 0:1], scr4[:, 0:1], scr4[:, 1:2], op=OP.add)
            nc.vector.tensor_tensor(scr4[:, 2:3], scr4[:, 2:3], scr4[:, 3:4], op=OP.add)
            nc.vector.tensor_tensor(scr4[:, 0:1], scr4[:, 0:1], scr4[:, 2:3], op=OP.add)
            nc.scalar.activation(
                rw[:, col : col + 1], scr4[:, 0:1], AF.Sqrt, bias=eps_t, scale=1.0 / DIN
            )
            nc.vector.reciprocal(rw[:, col : col + 1], rw[:, col : col + 1])
            nc.vector.tensor_reduce(
                sabs[:, col : col + 1], w_t, axis=AX.X, op=OP.add,
                apply_absolute_value=True,
            )
            nc.vector.tensor_scalar(
                traw[:, col : col + 1], sabs[:, col : col + 1], 0.5 / DIN, None,
                op0=OP.mult,
            )
            nc.vector.tensor_scalar(
                ntraw[:, col : col + 1], sabs[:, col : col + 1], -0.5 / DIN, None,
                op0=OP.mult,
            )
            # alpha = 0.5*mean|w|*rw*rs  (0.5: ternary e is in {-2,0,2})
            nc.vector.tensor_scalar(
                scr4[:, 1:2], sabs[:, col : col + 1], 0.5 / DIN, None, op0=OP.mult
            )
            nc.vector.tensor_tensor(
                scr4[:, 1:2], scr4[:, 1:2], rw[:, col : col + 1], op=OP.mult
            )
            alpha_dst = alpha_sh[:, col : col + 1] if col < 2 else \
                alpha_c[:, col - 2 : col - 1]
            nc.vector.tensor_tensor(
                alpha_dst, scr4[:, 1:2], rs_sh[:, col : col + 1], op=OP.mult
            )
            # ternary sign trick: e = Sign(w-t) + Sign(w+t) in {-2,0,2}
            a_t = abp.tile([P, DIN], bf16, name="at")
            b_t = abp.tile([P, DIN], bf16, name="bt")
            e_t = epp.tile([P, DIN], bf16, name="et")
            nc.scalar.activation(a_t, w_t, AF.Sign, bias=ntraw[:, col : col + 1])
            nc.scalar.activation(b_t, w_t, AF.Sign, bias=traw[:, col : col + 1])
            nc.vector.tensor_tensor(e_t, a_t, b_t, op=OP.add)
            e_sink(e_t)

        def tr_sink(dst_tile, ocol):
            def sink(e_t):
                for kk in range(KT // 4):
                    pt = ptp.tile([P, 2, 2, P], bf16, name="pt", bufs=1)
                    for q in range(4):
                        k = kk * 4 + q
                        nc.tensor.transpose(
                            pt[:, q // 2, q % 2, :],
                            e_t[:, k * P : (k + 1) * P], identbf,
                        )
                    dst = dst_tile[:, 2 * kk : 2 * kk + 2, :,
                                   ocol * P : (ocol + 1) * P]
                    if kk % 2 == 0:
                        nc.scalar.activation(dst, pt, AF.Copy)
                    else:
                        nc.vector.tensor_copy(dst, pt)
            return sink

        def bcast_alpha(j):
            aps = pmm.tile([P, P], f32, name="alps", bufs=1)
            nc.tensor.transpose(aps[0:1, :], alpha_c[:, j : j + 1], ident32)
            art = smp.tile([1, P], f32, name="art", bufs=2)
            nc.vector.tensor_copy(art, aps[0:1, :])
            nc.tensor.matmul(aps, ones1, art, start=True, stop=True)
            nc.vector.tensor_copy(alphab[:, j * P : (j + 1) * P], aps)

        def prep_all():
            # shard prep + AG launch first (collective runs in background)
            for i in range(2):
                prep_core(
                    w_d[bass.ds(pid * (2 * P) + i * P, P), :],
                    rs_d[bass.ds(pid * (2 * P) + i * P, P), :],
                    i, tr_sink(w2sh, i),
                )
            pay = dram.tile([P, PAIRS * 2 * 2 * P + 8], f8, name="pay")
            nc.gpsimd.dma_start(pay[:, : PAIRS * 2 * 2 * P], w2sh[:].opt())
            nc.gpsimd.dma_start(
                pay[:, PAIRS * 2 * 2 * P :], alpha_sh[:].bitcast(f8)
            )
            bout = dram.tile([NCORES * P, PAIRS * 2 * 2 * P + 8], f8, name="bout")
            nc.gpsimd.collective_compute(
                "AllGather",
                OP.bypass,
                replica_groups=[list(range(NCORES))],
                ins=[pay.opt()],
                outs=[bout.opt()],
            )
            # local chunks 0-1 prep overlaps the AG
            for j in range(8):
                prep_core(
                    w_d[j * P : (j + 1) * P, :],
                    rs_d[j * P : (j + 1) * P, :],
                    2 + j, tr_sink(w2a, j),
                )
                bcast_alpha(j)
            # AG readback (gpsimd queue, FIFO after the collective):
            # chunks 2-3 = shards of cores 4..7
            for c in range(4, NCORES):
                src = bout[c * P : (c + 1) * P, : PAIRS * 2 * 2 * P].rearrange(
                    "p (a b o) -> p a b o", a=PAIRS, b=2
                )
                nc.gpsimd.dma_start(
                    out=w2b[:, :, :, (c - 4) * 2 * P : (c - 3) * 2 * P], in_=src
                )
                nc.gpsimd.dma_start(
                    out=alpha_c[:, 2 * c : 2 * c + 2],
                    in_=bout[
                        c * P : (c + 1) * P, PAIRS * 2 * 2 * P :
                    ].bitcast(f32),
                )
            for j in range(8, KT):
                bcast_alpha(j)

        # ---- frontend: load x tile t, stats, transpose, fp8 main+residual ----
        def frontend(t):
            x_t = xtp.tile([P, DIN], f32, name="xt")
            nc.sync.dma_start(out=x_t, in_=x_d[t * P : (t + 1) * P, :])
            if "nofront" in ab:
                hi8 = hip.tile([P, PAIRS, 2, P], f8, name="hi8")
                r8 = rip.tile([P, PAIRS, 2, P], f8, name="r8")
                sclx = smp.tile([P, 1], f32, name="sclx", bufs=8)
                nc.vector.memset(hi8, 0.0)
                nc.vector.memset(r8, 0.0)
                nc.vector.tensor_reduce(
                    sclx, x_t[:, :4], axis=AX.X, op=OP.add,
                )
                return hi8, r8, sclx
            scr4 = smp.tile([P, 4], f32, name="scr4")
            for c in range(4):
                dump = pmm.tile([P, CH], f32, name="dump", bufs=1)
                nc.scalar.activation(
                    dump, x_t[:, c * CH : (c + 1) * CH], AF.Square,
                    accum_out=scr4[:, c : c + 1],
                )
            sclx = smp.tile([P, 1], f32, name="sclx", bufs=8)
            nc.vector.tensor_tensor(
                scr4[:, 0:1], scr4[:, 0:1], scr4[:, 1:2], op=OP.add
            )
            nc.vector.tensor_tensor(
                scr4[:, 2:3], scr4[:, 2:3], scr4[:, 3:4], op=OP.add
            )
            nc.vector.tensor_tensor(
                scr4[:, 0:1], scr4[:, 0:1], scr4[:, 2:3], op=OP.add
            )
            nc.scalar.activation(
                sclx, scr4[:, 0:1], AF.Sqrt, bias=eps_t, scale=1.0 / DIN
            )
            nc.vector.reciprocal(sclx, sclx)

            hi8 = hip.tile([P, PAIRS, 2, P], f8, name="hi8")
            r8 = rip.tile([P, PAIRS, 2, P], f8, name="r8")

            # DoubleRowSwInterleave expects the STATIONARY operand laid out
            # software-interleaved per 256-wide pair-block: flat[254-2m+ko]
            # holds the (k-tile 2jj+ko, out-row m) value.  The transposes
            # write PSUM through strided APs so the blocks are born in that
            # layout and every SBUF conversion below stays contiguous.
            def ilv_pt(pt_, jjh, ko):
                t_ap = pt_[:]
                return bass.AP(
                    tensor=t_ap.tensor,
                    offset=t_ap.offset + jjh * 256 + 254 + ko,
                    ap=[list(t_ap.ap[0]), [-2, P]],
                )

            for kk in range(KT // 4):
                pt = ptp.tile([P, 2, 2, P], f32, name="ptx")
                for q in range(4):
                    k = kk * 4 + q
                    nc.tensor.transpose(
                        ilv_pt(pt, q // 2, q % 2),
                        x_t[:, k * P : (k + 1) * P], ident32,
                    )
                hsl = hi8[:, 2 * kk : 2 * kk + 2, :, :]
                rsl = r8[:, 2 * kk : 2 * kk + 2, :, :]
                if g_one:
                    # main fp8 on ACT, residual (psum - main) on DVE
                    nc.scalar.activation(hsl, pt, AF.Copy)
                    if "noresid" not in ab:
                        nc.vector.tensor_tensor(rsl, pt, hsl, op=OP.subtract)
                else:
                    # general-g path: per-k-tile scale over the strided
                    # interleaved positions (slower; benchmark data has g=1)
                    for q in range(4):
                        k = kk * 4 + q
                        src = ilv_pt(pt, q // 2, q % 2)
                        hq_ap = bass.AP(
                            tensor=hsl.tensor,
                            offset=hsl.offset + (q // 2) * 256 + 254 + (q % 2),
                            ap=[list(hsl.ap[0]), [-2, P]],
                        )
                        rq_ap = bass.AP(
                            tensor=rsl.tensor,
                            offset=rsl.offset + (q // 2) * 256 + 254 + (q % 2),
                            ap=[list(rsl.ap[0]), [-2, P]],
                        )
                        nc.scalar.activation(
                            hq_ap, src, AF.Copy, scale=g_col[:, k : k + 1]
                        )
                        nc.vector.scalar_tensor_tensor(
                            out=rq_ap, in0=src,
                            scalar=g_col[:, k : k + 1], in1=hq_ap,
                            op0=OP.mult, op1=OP.subtract,
                        )
            return hi8, r8, sclx

        # ---- one (tile, chunk) unit: 16 DR matmuls + epilogue ----
        def unit(t, c, fr, ob):
            hi8, r8, sclx = fr
            pm = pmm.tile([P, CH], f32, name="pm")
            wsl = (
                (lambda jj: w2a[:, jj, :, c * CH : (c + 1) * CH])
                if c < 2
                else (lambda jj: w2b[:, jj, :, (c - 2) * CH : (c - 1) * CH])
            )
            for jj in range(PAIRS):
                nc.tensor.matmul(
                    pm, hi8[:, jj], wsl(jj),
                    start=(jj == 0),
                    stop=("noresid" in ab and jj == PAIRS - 1),
                    perf_mode=SW,
                )
            if "noresid" not in ab:
                for jj in range(PAIRS):
                    nc.tensor.matmul(
                        pm, r8[:, jj], wsl(jj),
                        start=False, stop=(jj == PAIRS - 1), perf_mode=SW,
                    )
            if "noepi" in ab:
                nc.vector.tensor_copy(ob[:, c * CH : (c + 1) * CH], pm)
            else:
                e1t = etp.tile([P, CH], bf16, name="e1t")
                nc.vector.scalar_tensor_tensor(
                    out=e1t, in0=pm, scalar=sclx,
                    in1=alphab[:, c * CH : (c + 1) * CH],
                    op0=OP.mult, op1=OP.mult,
                )
                if "nogpsadd" in ab:
                    nc.vector.tensor_copy(ob[:, c * CH : (c + 1) * CH], e1t)
                else:
                    nc.vector.tensor_tensor(
                        ob[:, c * CH : (c + 1) * CH], e1t,
                        biasb_bf[:, c * CH : (c + 1) * CH], op=OP.add,
                    )

        for rep in range(reps):
            fr = {0: frontend(0)}
            prep_all()
            obs, ndone = {}, {}
            for s, units in enumerate(steps):
                for t, c in units:
                    if t not in obs:
                        obs[t] = outp.tile([P, DOUT], bf16, name="ob", bufs=8)
                        ndone[t] = 0
                    unit(t, c, fr[t], obs[t])
                    ndone[t] += 1
                    if ndone[t] == NCH:
                        # one merged SWDGE store per s-tile
                        nc.gpsimd.dma_start(
                            out=o_d[t * P : (t + 1) * P, :], in_=obs.pop(t)
                        )
                # just-in-time frontends: emit those first used next step
                for t in range(ST):
                    if t not in fr and first_use.get(t) == s + 1:
                        fr[t] = frontend(t)
                for t in list(fr):
                    if last_use[t] <= s:
                        fr.pop(t)

    nc.compile()
    return nc


_CACHE = {}


def _get_nc(g_one=True):
    key = ("nc", g_one)
    if key not in _CACHE:
        _CACHE[key] = build_module(g_one=g_one)
    return _CACHE[key]


def kernel(**inputs) -> np.ndarray:
    g = np.asarray(inputs["g"], dtype=np.float32)
    nc = _get_nc(bool(np.all(g == 1.0)))
    x = np.ascontiguousarray(
        np.asarray(inputs["x"], dtype=np.float32).reshape(B * S, DIN)
    )
    shards = np.split(x, NCORES, axis=0)
    base = {
        k: np.ascontiguousarray(np.asarray(inputs[k], dtype=np.float32))
        for k in ("weight", "row_scale", "bias", "g")
    }
    in_maps = [{"x": shards[c], **base} for c in range(NCORES)]
    res = bass_utils.run_bass_kernel_spmd(nc, in_maps, list(range(NCORES)))
    out = np.concatenate(
        [np.asarray(res.results[c]["out"]) for c in range(NCORES)], axis=0
    )
    return out.reshape(B, S, DOUT).astype(np.float32)


# revision 21
# speedup vs baseline: 1.1557x; 1.1557x over previous
"""BitLinear (input-RMSNorm + ternary-quantized linear) on 8 TRN2 NeuronCores.

Math (reference):
  xn    = x * rsqrt(mean(x^2, -1) + eps) * g
  w     = weight * rsqrt(mean(weight^2, 1) + eps)          (row RMS norm)
  am    = mean(|w|, 1)
  w_q   = sign(w) * (|w| > 0.5*am)                          (ternary)
  out   = xn @ (w_q * am * row_scale).T + bias

Kernel strategy (per core, data-parallel over B*S rows; weight replicated):
  - fp8e4 DoubleRowSwInterleave matmuls (K=256/issue, measured ~94 ns per
    N=512 MM = ~2x the bf16 MAC rate).  The ternary weight {-1,0,1} is
    EXACT in fp8; x rides as fp8(main) + fp8(residual = x - fp8(x)), two
    accumulating GEMM chains into one PSUM bank, so the matmul path loses
    nothing vs bf16 accuracy (~2e-3 rel err vs the 2e-2 gate).
  - alpha = am*rw*row_scale stays f32: broadcast across partitions via a
    tiny PE ones-outer-product per weight tile, applied in the epilogue
    STT (psum*sclx)*alpha on DVE; bias added on gpsimd in bf16.
  - x row rsqrt (sclx) commutes with the matmul (applied per-partition in
    the epilogue STT); g is per-partition in the transposed domain and
    rides the PSUM->SBUF fp8 conversion copies (skipped when g == 1).
  - |w| > 0.5*mean|w| evaluated in the raw-weight domain (rsqrt cancels).
  - x transposed on PE in f32r (exact); main fp8 = ACT copy-cast from
    PSUM, residual = DVE (psum - main) -> fp8, written straight into the
    k-pair-interleaved layouts the DR matmuls consume.
  - Software-pipelined skew: 8 weight tiles prep ahead, 2 per step after;
    (s-tile, chunk) units gate only on the 4 preps their columns need.
  - DMA: x+w loads on the sync HWDGE ring; merged bf16 stores per s-tile
    issue from the gpsimd SWDGE queue.
"""

import sys

try:
    import concourse.bass  # noqa: F401
except ImportError:
    for _p in ("/opt/trn_rl_repo", "/root/.axon_site/_ro/trn_rl_repo"):
        if _p not in sys.path:
            sys.path.insert(0, _p)

from contextlib import ExitStack

import numpy as np

import concourse.bass as bass
import concourse.mybir as mybir
import concourse.tile as tile
from concourse import bacc, bass_utils
from concourse.masks import make_identity

B, S, DIN, DOUT = 4, 4096, 2048, 2048
NCORES = 8
SC = B * S // NCORES      # 2048 rows of x per core
P = 128
KT = DIN // P             # 16 k-tiles
PAIRS = KT // 2           # 8 k-pairs per DR chain
ST = SC // P              # 16 s-tiles per core
CH = 512                  # psum chunk (one bank of fp32)
NCH = DOUT // CH          # 4 chunks
EPS = 1e-8
EHEAD = 8                 # weight tiles prepped before the main loop
RPACE = 2                 # weight tiles prepped per early main-loop step

f32 = mybir.dt.float32
f32r = mybir.dt.float32r
bf16 = mybir.dt.bfloat16
f8 = mybir.dt.float8e4
AF = mybir.ActivationFunctionType
OP = mybir.AluOpType
AX = mybir.AxisListType
SW = mybir.MatmulPerfMode.DoubleRowSwInterleave


def _skew_schedule(ehead=None, rpace=None):
    """Greedy (tile, chunk) unit order: chunk c is eligible once its 4
    preps are done; units process oldest-tile-first, <=4 per step."""
    ehead = EHEAD if ehead is None else ehead
    rpace = RPACE if rpace is None else rpace
    steps = []
    pend = []
    npreps = ehead
    arrived = 0
    for s in range(ST + 4):
        while arrived < ST and arrived <= s + 1:
            pend += [(arrived, c) for c in range(NCH)]
            arrived += 1
        elig = sorted(u for u in pend if NCH * (u[1] + 1) <= npreps and u[0] <= s)
        take = elig[:NCH]
        for u in take:
            pend.remove(u)
        steps.append(take)
        npreps = min(KT, npreps + rpace)
    assert not pend, pend
    return steps


def _fr_lifetimes(steps):
    first_use, last_use = {}, {}
    for s, us in enumerate(steps):
        for t, _ in us:
            first_use.setdefault(t, s)
            last_use[t] = s
    alive = max(
        sum(1 for t in first_use if first_use[t] <= s <= last_use[t])
        for s in range(len(steps))
    )
    return first_use, last_use, alive


def build_module(reps=1, g_one=True, ehead=None, rpace=None, ablate=()):
    ab = set(ablate.split(",")) if isinstance(ablate, str) else set(ablate)
    nc = bacc.Bacc("TRN2", target_bir_lowering=False)
    x_d = nc.declare_dram_parameter("x", [SC, DIN], f32, isOutput=False)
    w_d = nc.declare_dram_parameter("weight", [DOUT, DIN], f32, isOutput=False)
    rs_d = nc.declare_dram_parameter("row_scale", [DOUT, 1], f32, isOutput=False)
    b_d = nc.declare_dram_parameter("bias", [DOUT], f32, isOutput=False)
    g_d = nc.declare_dram_parameter("g", [DIN], f32, isOutput=False)
    o_d = nc.declare_dram_parameter("out", [SC, DOUT], bf16, isOutput=True)

    with tile.TileContext(nc) as tc, ExitStack() as ctx:
        const = ctx.enter_context(tc.tile_pool(name="const", bufs=1))
        xtp = ctx.enter_context(tc.tile_pool(name="xtp", bufs=3))
        wtp = ctx.enter_context(tc.tile_pool(name="wtp", bufs=2))
        abp = ctx.enter_context(tc.tile_pool(name="abp", bufs=2))
        epp = ctx.enter_context(tc.tile_pool(name="epp", bufs=2))
        ehead_v = EHEAD if ehead is None else ehead
        rpace_v = RPACE if rpace is None else rpace
        steps = _skew_schedule(4, 2)
        first_use, last_use, alive = _fr_lifetimes(steps)
        hip = ctx.enter_context(tc.tile_pool(name="hip", bufs=alive + 1))
        rip = ctx.enter_context(tc.tile_pool(name="rip", bufs=alive + 1))
        outp = ctx.enter_context(tc.tile_pool(name="outp", bufs=3))
        etp = ctx.enter_context(tc.tile_pool(name="etp", bufs=4))
        smp = ctx.enter_context(tc.tile_pool(name="smp", bufs=4))
        pmm = ctx.enter_context(tc.tile_pool(name="pmm", bufs=3, space="PSUM"))
        ptp = ctx.enter_context(tc.tile_pool(name="ptp", bufs=2, space="PSUM"))
        dram = ctx.enter_context(tc.tile_pool(name="dram", bufs=2, space="DRAM"))

        # ---- constants ----
        w2a = const.tile([P, PAIRS, 2, CH], f8)    # ternary, chunk 0 (local prep)
        w2b = const.tile([P, PAIRS, 2, DOUT - CH], f8)  # chunks 1-3 (AllGather)
        biasb = const.tile([P, DOUT], f32)         # bias broadcast to all partitions
        biasb_bf = const.tile([P, DOUT], bf16)
        alphab = const.tile([P, DOUT], f32)        # alpha broadcast to all partitions
        ident32 = const.tile([P, P], f32)
        identbf = const.tile([P, P], bf16)
        make_identity(nc, ident32)
        nc.vector.tensor_copy(identbf, ident32)
        ident32r = ident32.bitcast(f32r)
        eps_t = const.tile([P, 1], f32)
        nc.vector.memset(eps_t, EPS)
        ones1 = const.tile([1, P], f32)
        nc.vector.memset(ones1, 1.0)
        g_col = const.tile([P, KT], f32)           # g[k], k = j*128+p -> [p, j]
        nc.gpsimd.dma_start(out=g_col, in_=g_d.rearrange("(j p) -> p j", p=P))

        # per-w-tile stats: cols 0-1 = AG shard, cols 2-9 = local chunks 0-1
        sabs = const.tile([P, 10], f32)
        rw = const.tile([P, 10], f32)
        traw = const.tile([P, 10], f32)
        ntraw = const.tile([P, 10], f32)
        alpha_sh = const.tile([P, 2], f32)
        alpha_c = const.tile([P, KT], f32)
        w2sh = const.tile([P, PAIRS, 2, 2 * P], f8)
        rs_sh = const.tile([P, 10], f32)

        # bias broadcast: DRAM [DOUT] replicated over 128 partitions
        bias_ap = b_d[:]
        nc.gpsimd.dma_start(
            out=biasb,
            in_=bass.AP(
                tensor=bias_ap.tensor, offset=bias_ap.offset,
                ap=[[0, P]] + list(bias_ap.ap),
            ),
        )
        nc.vector.tensor_copy(biasb_bf, biasb)

        # ---- weight prep ----
        # Shard: core c owns o rows [256c, 256c+256) -> ternary fp8 shard,
        # AllGathered for chunks 1-3 (w2b).  Chunk 0 (o < 512) is ALSO
        # prepped locally on every core (w2a) so the GEMM starts while the
        # collective is in flight.  AG readback rides the gpsimd queue so
        # the sync ring stays free for x loads.
        pid = nc.sync.partition_id()

        def prep_core(w_src_ap, rs_src_ap, col, e_sink):
            """stats + ternarize one o-tile; e_sink(e_t) consumes the ternary."""
            w_t = wtp.tile([P, DIN], f32, name="wt")
            nc.sync.dma_start(out=w_t, in_=w_src_ap)
            nc.sync.dma_start(out=rs_sh[:, col : col + 1], in_=rs_src_ap)
            scr4 = smp.tile([P, 4], f32, name="scr4")
            for c in range(4):
                dump = pmm.tile([P, CH], f32, name="dump", bufs=1)
                nc.scalar.activation(
                    dump, w_t[:, c * CH : (c + 1) * CH], AF.Square,
                    accum_out=scr4[:, c : c + 1],
                )
            nc.vector.tensor_tensor(scr4[:, 0:1], scr4[:, 0:1], scr4[:, 1:2], op=OP.add)
            nc.vector.tensor_tensor(scr4[:, 2:3], scr4[:, 2:3], scr4[:, 3:4], op=OP.add)
            nc.vector.tensor_tensor(scr4[:, 0:1], scr4[:, 0:1], scr4[:, 2:3], op=OP.add)
            nc.scalar.activation(
                rw[:, col : col + 1], scr4[:, 0:1], AF.Sqrt, bias=eps_t, scale=1.0 / DIN
            )
            nc.vector.reciprocal(rw[:, col : col + 1], rw[:, col : col + 1])
            nc.vector.tensor_reduce(
                sabs[:, col : col + 1], w_t, axis=AX.X, op=OP.add,
                apply_absolute_value=True,
            )
            nc.vector.tensor_scalar(
                traw[:, col : col + 1], sabs[:, col : col + 1], 0.5 / DIN, None,
                op0=OP.mult,
            )
            nc.vector.tensor_scalar(
                ntraw[:, col : col + 1], sabs[:, col : col + 1], -0.5 / DIN, None,
                op0=OP.mult,
            )
            # alpha = 0.5*mean|w|*rw*rs  (0.5: ternary e is in {-2,0,2})
            nc.vector.tensor_scalar(
                scr4[:, 1:2], sabs[:, col : col + 1], 0.5 / DIN, None, op0=OP.mult
            )
            nc.vector.tensor_tensor(
                scr4[:, 1:2], scr4[:, 1:2], rw[:, col : col + 1], op=OP.mult
            )
            alpha_dst = alpha_sh[:, col : col + 1] if col < 2 else \
                alpha_c[:, col - 2 : col - 1]
            nc.vector.tensor_tensor(
                alpha_dst, scr4[:, 1:2], rs_sh[:, col : col + 1], op=OP.mult
            )
            # ternary sign trick: e = Sign(w-t) + Sign(w+t) in {-2,0,2}
            a_t = abp.tile([P, DIN], bf16, name="at")
            b_t = abp.tile([P, DIN], bf16, name="bt")
            e_t = epp.tile([P, DIN], bf16, name="et")
            nc.scalar.activation(a_t, w_t, AF.Sign, bias=ntraw[:, col : col + 1])
            nc.scalar.activation(b_t, w_t, AF.Sign, bias=traw[:, col : col + 1])
            nc.vector.tensor_tensor(e_t, a_t, b_t, op=OP.add)
            e_sink(e_t)

        def tr_sink(dst_tile, ocol):
            def sink(e_t):
                for kk in range(KT // 4):
                    pt = ptp.tile([P, 2, 2, P], bf16, name="pt", bufs=1)
                    for q in range(4):
                        k = kk * 4 + q
                        nc.tensor.transpose(
                            pt[:, q // 2, q % 2, :],
                            e_t[:, k * P : (k + 1) * P], identbf,
                        )
                    dst = dst_tile[:, 2 * kk : 2 * kk + 2, :,
                                   ocol * P : (ocol + 1) * P]
                    if kk % 2 == 0:
                        nc.scalar.activation(dst, pt, AF.Copy)
                    else:
                        nc.vector.tensor_copy(dst, pt)
            return sink

        def bcast_alpha(j):
            aps = pmm.tile([P, P], f32, name="alps", bufs=1)
            nc.tensor.transpose(aps[0:1, :], alpha_c[:, j : j + 1], ident32)
            art = smp.tile([1, P], f32, name="art", bufs=2)
            nc.vector.tensor_copy(art, aps[0:1, :])
            nc.tensor.matmul(aps, ones1, art, start=True, stop=True)
            nc.vector.tensor_copy(alphab[:, j * P : (j + 1) * P], aps)

        def prep_all():
            # shard prep + AG launch first (collective runs in background)
            for i in range(2):
                prep_core(
                    w_d[bass.ds(pid * (2 * P) + i * P, P), :],
                    rs_d[bass.ds(pid * (2 * P) + i * P, P), :],
                    i, tr_sink(w2sh, i),
                )
            pay = dram.tile([P, PAIRS * 2 * 2 * P + 8], f8, name="pay")
            nc.gpsimd.dma_start(pay[:, : PAIRS * 2 * 2 * P], w2sh[:].opt())
            nc.gpsimd.dma_start(
                pay[:, PAIRS * 2 * 2 * P :], alpha_sh[:].bitcast(f8)
            )
            bout = dram.tile([NCORES * P, PAIRS * 2 * 2 * P + 8], f8, name="bout")
            nc.gpsimd.collective_compute(
                "AllGather",
                OP.bypass,
                replica_groups=[list(range(NCORES))],
                ins=[pay.opt()],
                outs=[bout.opt()],
            )
            # local chunk-0 prep overlaps the AG
            for j in range(4):
                prep_core(
                    w_d[j * P : (j + 1) * P, :],
                    rs_d[j * P : (j + 1) * P, :],
                    2 + j, tr_sink(w2a, j),
                )
                bcast_alpha(j)
            # AG readback (gpsimd queue, FIFO after the collective):
            # chunks 1-3 = shards of cores 2..7
            for c in range(2, NCORES):
                src = bout[c * P : (c + 1) * P, : PAIRS * 2 * 2 * P].rearrange(
                    "p (a b o) -> p a b o", a=PAIRS, b=2
                )
                nc.gpsimd.dma_start(
                    out=w2b[:, :, :, (c - 2) * 2 * P : (c - 1) * 2 * P], in_=src
                )
                nc.gpsimd.dma_start(
                    out=alpha_c[:, 2 * c : 2 * c + 2],
                    in_=bout[
                        c * P : (c + 1) * P, PAIRS * 2 * 2 * P :
                    ].bitcast(f32),
                )
            for j in range(4, KT):
                bcast_alpha(j)

        # ---- frontend: load x tile t, stats, transpose, fp8 main+residual ----
        def frontend(t):
            x_t = xtp.tile([P, DIN], f32, name="xt")
            nc.sync.dma_start(out=x_t, in_=x_d[t * P : (t + 1) * P, :])
            if "nofront" in ab:
                hi8 = hip.tile([P, PAIRS, 2, P], f8, name="hi8")
                r8 = rip.tile([P, PAIRS, 2, P], f8, name="r8")
                sclx = smp.tile([P, 1], f32, name="sclx", bufs=8)
                nc.vector.memset(hi8, 0.0)
                nc.vector.memset(r8, 0.0)
                nc.vector.tensor_reduce(
                    sclx, x_t[:, :4], axis=AX.X, op=OP.add,
                )
                return hi8, r8, sclx
            scr4 = smp.tile([P, 4], f32, name="scr4")
            for c in range(4):
                dump = pmm.tile([P, CH], f32, name="dump", bufs=1)
                nc.scalar.activation(
                    dump, x_t[:, c * CH : (c + 1) * CH], AF.Square,
                    accum_out=scr4[:, c : c + 1],
                )
            sclx = smp.tile([P, 1], f32, name="sclx", bufs=8)
            nc.vector.tensor_tensor(
                scr4[:, 0:1], scr4[:, 0:1], scr4[:, 1:2], op=OP.add
            )
            nc.vector.tensor_tensor(
                scr4[:, 2:3], scr4[:, 2:3], scr4[:, 3:4], op=OP.add
            )
            nc.vector.tensor_tensor(
                scr4[:, 0:1], scr4[:, 0:1], scr4[:, 2:3], op=OP.add
            )
            nc.scalar.activation(
                sclx, scr4[:, 0:1], AF.Sqrt, bias=eps_t, scale=1.0 / DIN
            )
            nc.vector.reciprocal(sclx, sclx)

            hi8 = hip.tile([P, PAIRS, 2, P], f8, name="hi8")
            r8 = rip.tile([P, PAIRS, 2, P], f8, name="r8")

            # DoubleRowSwInterleave expects the STATIONARY operand laid out
            # software-interleaved per 256-wide pair-block: flat[254-2m+ko]
            # holds the (k-tile 2jj+ko, out-row m) value.  The transposes
            # write PSUM through strided APs so the blocks are born in that
            # layout and every SBUF conversion below stays contiguous.
            def ilv_pt(pt_, jjh, ko):
                t_ap = pt_[:]
                return bass.AP(
                    tensor=t_ap.tensor,
                    offset=t_ap.offset + jjh * 256 + 254 + ko,
                    ap=[list(t_ap.ap[0]), [-2, P]],
                )

            for kk in range(KT // 4):
                pt = ptp.tile([P, 2, 2, P], f32, name="ptx")
                for q in range(4):
                    k = kk * 4 + q
                    nc.tensor.transpose(
                        ilv_pt(pt, q // 2, q % 2),
                        x_t[:, k * P : (k + 1) * P], ident32,
                    )
                hsl = hi8[:, 2 * kk : 2 * kk + 2, :, :]
                rsl = r8[:, 2 * kk : 2 * kk + 2, :, :]
                if g_one:
                    # main fp8 on ACT, residual (psum - main) on DVE
                    nc.scalar.activation(hsl, pt, AF.Copy)
                    if "noresid" not in ab:
                        nc.vector.tensor_tensor(rsl, pt, hsl, op=OP.subtract)
                else:
                    # general-g path: per-k-tile scale over the strided
                    # interleaved positions (slower; benchmark data has g=1)
                    for q in range(4):
                        k = kk * 4 + q
                        src = ilv_pt(pt, q // 2, q % 2)
                        hq_ap = bass.AP(
                            tensor=hsl.tensor,
                            offset=hsl.offset + (q // 2) * 256 + 254 + (q % 2),
                            ap=[list(hsl.ap[0]), [-2, P]],
                        )
                        rq_ap = bass.AP(
                            tensor=rsl.tensor,
                            offset=rsl.offset + (q // 2) * 256 + 254 + (q % 2),
                            ap=[list(rsl.ap[0]), [-2, P]],
                        )
                        nc.scalar.activation(
                            hq_ap, src, AF.Copy, scale=g_col[:, k : k + 1]
                        )
                        nc.vector.scalar_tensor_tensor(
                            out=rq_ap, in0=src,
                            scalar=g_col[:, k : k + 1], in1=hq_ap,
                            op0=OP.mult, op1=OP.subtract,
                        )
            return hi8, r8, sclx

        # ---- one (tile, chunk) unit: 16 DR matmuls + epilogue ----
        def unit(t, c, fr, ob):
            hi8, r8, sclx = fr
            pm = pmm.tile([P, CH], f32, name="pm")
            wsl = (
                (lambda jj: w2a[:, jj, :, :])
                if c == 0
                else (lambda jj: w2b[:, jj, :, (c - 1) * CH : c * CH])
            )
            for jj in range(PAIRS):
                nc.tensor.matmul(
                    pm, hi8[:, jj], wsl(jj),
                    start=(jj == 0),
                    stop=("noresid" in ab and jj == PAIRS - 1),
                    perf_mode=SW,
                )
            if "noresid" not in ab:
                for jj in range(PAIRS):
                    nc.tensor.matmul(
                        pm, r8[:, jj], wsl(jj),
                        start=False, stop=(jj == PAIRS - 1), perf_mode=SW,
                    )
            if "noepi" in ab:
                nc.vector.tensor_copy(ob[:, c * CH : (c + 1) * CH], pm)
            else:
                e1t = etp.tile([P, CH], bf16, name="e1t")
                nc.vector.scalar_tensor_tensor(
                    out=e1t, in0=pm, scalar=sclx,
                    in1=alphab[:, c * CH : (c + 1) * CH],
                    op0=OP.mult, op1=OP.mult,
                )
                if "nogpsadd" in ab:
                    nc.vector.tensor_copy(ob[:, c * CH : (c + 1) * CH], e1t)
                else:
                    nc.vector.tensor_tensor(
                        ob[:, c * CH : (c + 1) * CH], e1t,
                        biasb_bf[:, c * CH : (c + 1) * CH], op=OP.add,
                    )

        for rep in range(reps):
            fr = {0: frontend(0)}
            prep_all()
            obs, ndone = {}, {}
            for s, units in enumerate(steps):
                for t, c in units:
                    if t not in obs:
                        obs[t] = outp.tile([P, DOUT], bf16, name="ob", bufs=5)
                        ndone[t] = 0
                    unit(t, c, fr[t], obs[t])
                    ndone[t] += 1
                    if ndone[t] == NCH:
                        # one merged SWDGE store per s-tile
                        nc.gpsimd.dma_start(
                            out=o_d[t * P : (t + 1) * P, :], in_=obs.pop(t)
                        )
                # just-in-time frontends: emit those first used next step
                for t in range(ST):
                    if t not in fr and first_use.get(t) == s + 1:
                        fr[t] = frontend(t)
                for t in list(fr):
                    if last_use[t] <= s:
                        fr.pop(t)

    nc.compile()
    return nc


_CACHE = {}


def _get_nc(g_one=True):
    key = ("nc", g_one)
    if key not in _CACHE:
        _CACHE[key] = build_module(g_one=g_one)
    return _CACHE[key]


def kernel(**inputs) -> np.ndarray:
    g = np.asarray(inputs["g"], dtype=np.float32)
    nc = _get_nc(bool(np.all(g == 1.0)))
    x = np.ascontiguousarray(
        np.asarray(inputs["x"], dtype=np.float32).reshape(B * S, DIN)
    )
    shards = np.split(x, NCORES, axis=0)
    base = {
        k: np.ascontiguousarray(np.asarray(inputs[k], dtype=np.float32))
        for k in ("weight", "row_scale", "bias", "g")
    }
    in_maps = [{"x": shards[c], **base} for c in range(NCORES)]
    res = bass_utils.run_bass_kernel_spmd(nc, in_maps, list(range(NCORES)))
    out = np.concatenate(
        [np.asarray(res.results[c]["out"]) for c in range(NCORES)], axis=0
    )
    return out.reshape(B, S, DOUT).astype(np.float32)


# revision 22
# speedup vs baseline: 1.2903x; 1.1164x over previous
"""BitLinear (input-RMSNorm + ternary-quantized linear) on 8 TRN2 NeuronCores.

Math (reference):
  xn    = x * rsqrt(mean(x^2, -1) + eps) * g
  w     = weight * rsqrt(mean(weight^2, 1) + eps)          (row RMS norm)
  am    = mean(|w|, 1)
  w_q   = sign(w) * (|w| > 0.5*am)                          (ternary)
  out   = xn @ (w_q * am * row_scale).T + bias

Kernel strategy (per core, data-parallel over B*S rows; weight replicated):
  - fp8e4 DoubleRowSwInterleave matmuls (K=256/issue, measured ~94 ns per
    N=512 MM = ~2x the bf16 MAC rate).  The ternary weight {-1,0,1} is
    EXACT in fp8; x rides as fp8(main) + fp8(residual = x - fp8(x)), two
    accumulating GEMM chains into one PSUM bank, so the matmul path loses
    nothing vs bf16 accuracy (~2e-3 rel err vs the 2e-2 gate).
  - alpha = am*rw*row_scale stays f32: broadcast across partitions via a
    tiny PE ones-outer-product per weight tile, applied in the epilogue
    STT (psum*sclx)*alpha on DVE; bias added on gpsimd in bf16.
  - x row rsqrt (sclx) commutes with the matmul (applied per-partition in
    the epilogue STT); g is per-partition in the transposed domain and
    rides the PSUM->SBUF fp8 conversion copies (skipped when g == 1).
  - |w| > 0.5*mean|w| evaluated in the raw-weight domain (rsqrt cancels).
  - x transposed on PE in f32r (exact); main fp8 = ACT copy-cast from
    PSUM, residual = DVE (psum - main) -> fp8, written straight into the
    k-pair-interleaved layouts the DR matmuls consume.
  - Software-pipelined skew: 8 weight tiles prep ahead, 2 per step after;
    (s-tile, chunk) units gate only on the 4 preps their columns need.
  - DMA: x+w loads on the sync HWDGE ring; merged bf16 stores per s-tile
    issue from the gpsimd SWDGE queue.
"""

import sys

try:
    import concourse.bass  # noqa: F401
except ImportError:
    for _p in ("/opt/trn_rl_repo", "/root/.axon_site/_ro/trn_rl_repo"):
        if _p not in sys.path:
            sys.path.insert(0, _p)

from contextlib import ExitStack

import numpy as np

import concourse.bass as bass
import concourse.mybir as mybir
import concourse.tile as tile
from concourse import bacc, bass_utils
from concourse.masks import make_identity

B, S, DIN, DOUT = 4, 4096, 2048, 2048
NCORES = 8
SC = B * S // NCORES      # 2048 rows of x per core
P = 128
KT = DIN // P             # 16 k-tiles
PAIRS = KT // 2           # 8 k-pairs per DR chain
ST = SC // P              # 16 s-tiles per core
CH = 512                  # psum chunk (one bank of fp32)
NCH = DOUT // CH          # 4 chunks
EPS = 1e-8
EHEAD = 8                 # weight tiles prepped before the main loop
RPACE = 2                 # weight tiles prepped per early main-loop step

f32 = mybir.dt.float32
f32r = mybir.dt.float32r
bf16 = mybir.dt.bfloat16
f8 = mybir.dt.float8e4
AF = mybir.ActivationFunctionType
OP = mybir.AluOpType
AX = mybir.AxisListType
SW = mybir.MatmulPerfMode.DoubleRowSwInterleave


def _skew_schedule(ehead=None, rpace=None):
    """Greedy (tile, chunk) unit order: chunk c is eligible once its 4
    preps are done; units process oldest-tile-first, <=4 per step."""
    ehead = EHEAD if ehead is None else ehead
    rpace = RPACE if rpace is None else rpace
    steps = []
    pend = []
    npreps = ehead
    arrived = 0
    for s in range(ST + 4):
        while arrived < ST and arrived <= s + 1:
            pend += [(arrived, c) for c in range(NCH)]
            arrived += 1
        elig = sorted(u for u in pend if NCH * (u[1] + 1) <= npreps and u[0] <= s)
        take = elig[:NCH]
        for u in take:
            pend.remove(u)
        steps.append(take)
        npreps = min(KT, npreps + rpace)
    assert not pend, pend
    return steps


def _fr_lifetimes(steps):
    first_use, last_use = {}, {}
    for s, us in enumerate(steps):
        for t, _ in us:
            first_use.setdefault(t, s)
            last_use[t] = s
    alive = max(
        sum(1 for t in first_use if first_use[t] <= s <= last_use[t])
        for s in range(len(steps))
    )
    return first_use, last_use, alive


def build_module(reps=1, g_one=True, ehead=None, rpace=None, ablate=()):
    ab = set(ablate.split(",")) if isinstance(ablate, str) else set(ablate)
    nc = bacc.Bacc("TRN2", target_bir_lowering=False)
    x_d = nc.declare_dram_parameter("x", [SC, DIN], f32, isOutput=False)
    w_d = nc.declare_dram_parameter("weight", [DOUT, DIN], f32, isOutput=False)
    rs_d = nc.declare_dram_parameter("row_scale", [DOUT, 1], f32, isOutput=False)
    b_d = nc.declare_dram_parameter("bias", [DOUT], f32, isOutput=False)
    g_d = nc.declare_dram_parameter("g", [DIN], f32, isOutput=False)
    o_d = nc.declare_dram_parameter("out", [SC, DOUT], bf16, isOutput=True)

    with tile.TileContext(nc) as tc, ExitStack() as ctx:
        const = ctx.enter_context(tc.tile_pool(name="const", bufs=1))
        xtp = ctx.enter_context(tc.tile_pool(name="xtp", bufs=3))
        wtp = ctx.enter_context(tc.tile_pool(name="wtp", bufs=2))
        abp = ctx.enter_context(tc.tile_pool(name="abp", bufs=2))
        epp = ctx.enter_context(tc.tile_pool(name="epp", bufs=2))
        ehead_v = EHEAD if ehead is None else ehead
        rpace_v = RPACE if rpace is None else rpace
        steps = _skew_schedule(4, 2)
        first_use, last_use, alive = _fr_lifetimes(steps)
        hip = ctx.enter_context(tc.tile_pool(name="hip", bufs=alive + 1))
        rip = ctx.enter_context(tc.tile_pool(name="rip", bufs=alive + 1))
        outp = ctx.enter_context(tc.tile_pool(name="outp", bufs=3))
        etp = ctx.enter_context(tc.tile_pool(name="etp", bufs=4))
        smp = ctx.enter_context(tc.tile_pool(name="smp", bufs=4))
        pmm = ctx.enter_context(tc.tile_pool(name="pmm", bufs=3, space="PSUM"))
        ptp = ctx.enter_context(tc.tile_pool(name="ptp", bufs=2, space="PSUM"))
        dram = ctx.enter_context(tc.tile_pool(name="dram", bufs=2, space="DRAM"))

        # ---- constants ----
        w2a = const.tile([P, PAIRS, 2, CH], f8)    # ternary, chunk 0 (local prep)
        w2b = const.tile([P, PAIRS, 2, DOUT - CH], f8)  # chunks 1-3 (AllGather)
        biasb = const.tile([P, DOUT], f32)         # bias broadcast to all partitions
        biasb_bf = const.tile([P, DOUT], bf16)
        alphab = const.tile([P, DOUT], f32)        # alpha broadcast to all partitions
        ident32 = const.tile([P, P], f32)
        identbf = const.tile([P, P], bf16)
        make_identity(nc, ident32)
        nc.vector.tensor_copy(identbf, ident32)
        ident32r = ident32.bitcast(f32r)
        eps_t = const.tile([P, 1], f32)
        nc.vector.memset(eps_t, EPS)
        ones1 = const.tile([1, P], f32)
        nc.vector.memset(ones1, 1.0)
        g_col = const.tile([P, KT], f32)           # g[k], k = j*128+p -> [p, j]
        nc.gpsimd.dma_start(out=g_col, in_=g_d.rearrange("(j p) -> p j", p=P))

        # per-w-tile stats: cols 0-1 = AG shard, cols 2-9 = local chunks 0-1
        sabs = const.tile([P, 10], f32)
        rw = const.tile([P, 10], f32)
        traw = const.tile([P, 10], f32)
        ntraw = const.tile([P, 10], f32)
        alpha_sh = const.tile([P, 2], f32)
        alpha_c = const.tile([P, KT], f32)
        w2sh = const.tile([P, PAIRS, 2, 2 * P], f8)
        rs_sh = const.tile([P, 10], f32)

        # bias broadcast: DRAM [DOUT] replicated over 128 partitions
        bias_ap = b_d[:]
        nc.gpsimd.dma_start(
            out=biasb,
            in_=bass.AP(
                tensor=bias_ap.tensor, offset=bias_ap.offset,
                ap=[[0, P]] + list(bias_ap.ap),
            ),
        )
        nc.vector.tensor_copy(biasb_bf, biasb)

        # ---- weight prep ----
        # Shard: core c owns o rows [256c, 256c+256) -> ternary fp8 shard,
        # AllGathered for chunks 1-3 (w2b).  Chunk 0 (o < 512) is ALSO
        # prepped locally on every core (w2a) so the GEMM starts while the
        # collective is in flight.  AG readback rides the gpsimd queue so
        # the sync ring stays free for x loads.
        pid = nc.sync.partition_id()

        def prep_core(w_src_ap, rs_src_ap, col, e_sink):
            """stats + ternarize one o-tile; e_sink(e_t) consumes the ternary."""
            w_t = wtp.tile([P, DIN], f32, name="wt")
            nc.sync.dma_start(out=w_t, in_=w_src_ap)
            nc.sync.dma_start(out=rs_sh[:, col : col + 1], in_=rs_src_ap)
            scr4 = smp.tile([P, 4], f32, name="scr4")
            for c in range(4):
                dump = pmm.tile([P, CH], f32, name="dump", bufs=1)
                nc.scalar.activation(
                    dump, w_t[:, c * CH : (c + 1) * CH], AF.Square,
                    accum_out=scr4[:, c : c + 1],
                )
            nc.vector.tensor_tensor(scr4[:, 0:1], scr4[:, 0:1], scr4[:, 1:2], op=OP.add)
            nc.vector.tensor_tensor(scr4[:, 2:3], scr4[:, 2:3], scr4[:, 3:4], op=OP.add)
            nc.vector.tensor_tensor(scr4[:, 0:1], scr4[:, 0:1], scr4[:, 2:3], op=OP.add)
            nc.scalar.activation(
                rw[:, col : col + 1], scr4[:, 0:1], AF.Sqrt, bias=eps_t, scale=1.0 / DIN
            )
            nc.vector.reciprocal(rw[:, col : col + 1], rw[:, col : col + 1])
            nc.vector.tensor_reduce(
                sabs[:, col : col + 1], w_t, axis=AX.X, op=OP.add,
                apply_absolute_value=True,
            )
            nc.vector.tensor_scalar(
                traw[:, col : col + 1], sabs[:, col : col + 1], 0.5 / DIN, None,
                op0=OP.mult,
            )
            nc.vector.tensor_scalar(
                ntraw[:, col : col + 1], sabs[:, col : col + 1], -0.5 / DIN, None,
                op0=OP.mult,
            )
            # alpha = 0.5*mean|w|*rw*rs  (0.5: ternary e is in {-2,0,2})
            nc.vector.tensor_scalar(
                scr4[:, 1:2], sabs[:, col : col + 1], 0.5 / DIN, None, op0=OP.mult
            )
            nc.vector.tensor_tensor(
                scr4[:, 1:2], scr4[:, 1:2], rw[:, col : col + 1], op=OP.mult
            )
            alpha_dst = alpha_sh[:, col : col + 1] if col < 2 else \
                alpha_c[:, col - 2 : col - 1]
            nc.vector.tensor_tensor(
                alpha_dst, scr4[:, 1:2], rs_sh[:, col : col + 1], op=OP.mult
            )
            # ternary sign trick: e = Sign(w-t) + Sign(w+t) in {-2,0,2}
            a_t = abp.tile([P, DIN], bf16, name="at")
            b_t = abp.tile([P, DIN], bf16, name="bt")
            e_t = epp.tile([P, DIN], bf16, name="et")
            nc.scalar.activation(a_t, w_t, AF.Sign, bias=ntraw[:, col : col + 1])
            nc.scalar.activation(b_t, w_t, AF.Sign, bias=traw[:, col : col + 1])
            nc.vector.tensor_tensor(e_t, a_t, b_t, op=OP.add)
            e_sink(e_t)

        def tr_sink(dst_tile, ocol):
            def sink(e_t):
                for kk in range(KT // 4):
                    pt = ptp.tile([P, 2, 2, P], bf16, name="pt", bufs=1)
                    for q in range(4):
                        k = kk * 4 + q
                        nc.tensor.transpose(
                            pt[:, q // 2, q % 2, :],
                            e_t[:, k * P : (k + 1) * P], identbf,
                        )
                    dst = dst_tile[:, 2 * kk : 2 * kk + 2, :,
                                   ocol * P : (ocol + 1) * P]
                    if kk % 2 == 0:
                        nc.scalar.activation(dst, pt, AF.Copy)
                    else:
                        nc.vector.tensor_copy(dst, pt)
            return sink

        def bcast_alpha(j):
            aps = pmm.tile([P, P], f32, name="alps", bufs=1)
            nc.tensor.transpose(aps[0:1, :], alpha_c[:, j : j + 1], ident32)
            art = smp.tile([1, P], f32, name="art", bufs=2)
            nc.vector.tensor_copy(art, aps[0:1, :])
            nc.tensor.matmul(aps, ones1, art, start=True, stop=True)
            nc.vector.tensor_copy(alphab[:, j * P : (j + 1) * P], aps)

        def prep_all():
            # shard prep + AG launch first (collective runs in background)
            for i in range(2):
                prep_core(
                    w_d[bass.ds(pid * (2 * P) + i * P, P), :],
                    rs_d[bass.ds(pid * (2 * P) + i * P, P), :],
                    i, tr_sink(w2sh, i),
                )
            pay = dram.tile([P, PAIRS * 2 * 2 * P + 8], f8, name="pay")
            nc.gpsimd.dma_start(pay[:, : PAIRS * 2 * 2 * P], w2sh[:].opt())
            nc.gpsimd.dma_start(
                pay[:, PAIRS * 2 * 2 * P :], alpha_sh[:].bitcast(f8)
            )
            bout = dram.tile([NCORES * P, PAIRS * 2 * 2 * P + 8], f8, name="bout")
            nc.gpsimd.collective_compute(
                "AllGather",
                OP.bypass,
                replica_groups=[list(range(NCORES))],
                ins=[pay.opt()],
                outs=[bout.opt()],
            )
            # local chunk-0 prep overlaps the AG
            for j in range(4):
                prep_core(
                    w_d[j * P : (j + 1) * P, :],
                    rs_d[j * P : (j + 1) * P, :],
                    2 + j, tr_sink(w2a, j),
                )
                bcast_alpha(j)
            # AG readback (gpsimd queue, FIFO after the collective):
            # chunks 1-3 = shards of cores 2..7
            for c in range(2, NCORES):
                src = bout[c * P : (c + 1) * P, : PAIRS * 2 * 2 * P].rearrange(
                    "p (a b o) -> p a b o", a=PAIRS, b=2
                )
                nc.gpsimd.dma_start(
                    out=w2b[:, :, :, (c - 2) * 2 * P : (c - 1) * 2 * P], in_=src
                )
                nc.gpsimd.dma_start(
                    out=alpha_c[:, 2 * c : 2 * c + 2],
                    in_=bout[
                        c * P : (c + 1) * P, PAIRS * 2 * 2 * P :
                    ].bitcast(f32),
                )
            for j in range(4, KT):
                bcast_alpha(j)

        # ---- frontend: load x tile t, stats, transpose, fp8 main+residual ----
        def frontend(t):
            x_t = xtp.tile([P, DIN], f32, name="xt")
            nc.sync.dma_start(out=x_t, in_=x_d[t * P : (t + 1) * P, :])
            if "nofront" in ab:
                hi8 = hip.tile([P, PAIRS, 2, P], f8, name="hi8")
                r8 = rip.tile([P, PAIRS, 2, P], f8, name="r8")
                sclx = smp.tile([P, 1], f32, name="sclx", bufs=8)
                nc.vector.memset(hi8, 0.0)
                nc.vector.memset(r8, 0.0)
                nc.vector.tensor_reduce(
                    sclx, x_t[:, :4], axis=AX.X, op=OP.add,
                )
                return hi8, r8, sclx
            scr4 = smp.tile([P, 4], f32, name="scr4")
            for c in range(4):
                dump = pmm.tile([P, CH], f32, name="dump", bufs=1)
                nc.scalar.activation(
                    dump, x_t[:, c * CH : (c + 1) * CH], AF.Square,
                    accum_out=scr4[:, c : c + 1],
                )
            sclx = smp.tile([P, 1], f32, name="sclx", bufs=8)
            nc.vector.tensor_tensor(
                scr4[:, 0:1], scr4[:, 0:1], scr4[:, 1:2], op=OP.add
            )
            nc.vector.tensor_tensor(
                scr4[:, 2:3], scr4[:, 2:3], scr4[:, 3:4], op=OP.add
            )
            nc.vector.tensor_tensor(
                scr4[:, 0:1], scr4[:, 0:1], scr4[:, 2:3], op=OP.add
            )
            nc.scalar.activation(
                sclx, scr4[:, 0:1], AF.Sqrt, bias=eps_t, scale=1.0 / DIN
            )
            nc.vector.reciprocal(sclx, sclx)

            hi8 = hip.tile([P, PAIRS, 2, P], f8, name="hi8")
            r8 = rip.tile([P, PAIRS, 2, P], f8, name="r8")

            # DoubleRowSwInterleave expects the STATIONARY operand laid out
            # software-interleaved per 256-wide pair-block: flat[254-2m+ko]
            # holds the (k-tile 2jj+ko, out-row m) value.  The transposes
            # write PSUM through strided APs so the blocks are born in that
            # layout and every SBUF conversion below stays contiguous.
            def ilv_pt(pt_, jjh, ko):
                t_ap = pt_[:]
                return bass.AP(
                    tensor=t_ap.tensor,
                    offset=t_ap.offset + jjh * 256 + 254 + ko,
                    ap=[list(t_ap.ap[0]), [-2, P]],
                )

            for kk in range(KT // 4):
                pt = ptp.tile([P, 2, 2, P], f32, name="ptx")
                for q in range(4):
                    k = kk * 4 + q
                    nc.tensor.transpose(
                        ilv_pt(pt, q // 2, q % 2),
                        x_t[:, k * P : (k + 1) * P], ident32,
                    )
                hsl = hi8[:, 2 * kk : 2 * kk + 2, :, :]
                rsl = r8[:, 2 * kk : 2 * kk + 2, :, :]
                if g_one:
                    # main fp8 on ACT, residual (psum - main) on DVE
                    nc.scalar.activation(hsl, pt, AF.Copy)
                    if "noresid" not in ab:
                        nc.vector.tensor_tensor(rsl, pt, hsl, op=OP.subtract)
                else:
                    # general-g path: per-k-tile scale over the strided
                    # interleaved positions (slower; benchmark data has g=1)
                    for q in range(4):
                        k = kk * 4 + q
                        src = ilv_pt(pt, q // 2, q % 2)
                        hq_ap = bass.AP(
                            tensor=hsl.tensor,
                            offset=hsl.offset + (q // 2) * 256 + 254 + (q % 2),
                            ap=[list(hsl.ap[0]), [-2, P]],
                        )
                        rq_ap = bass.AP(
                            tensor=rsl.tensor,
                            offset=rsl.offset + (q // 2) * 256 + 254 + (q % 2),
                            ap=[list(rsl.ap[0]), [-2, P]],
                        )
                        nc.scalar.activation(
                            hq_ap, src, AF.Copy, scale=g_col[:, k : k + 1]
                        )
                        nc.vector.scalar_tensor_tensor(
                            out=rq_ap, in0=src,
                            scalar=g_col[:, k : k + 1], in1=hq_ap,
                            op0=OP.mult, op1=OP.subtract,
                        )
            return hi8, r8, sclx

        # ---- one (tile, chunk) unit: 16 DR matmuls + epilogue ----
        def unit(t, c, fr, ob):
            hi8, r8, sclx = fr
            pm = pmm.tile([P, CH], f32, name="pm")
            wsl = (
                (lambda jj: w2a[:, jj, :, :])
                if c == 0
                else (lambda jj: w2b[:, jj, :, (c - 1) * CH : c * CH])
            )
            # hi/r8 interleaved per k-pair so the moving operand stays
            # constant across each 2-MM run (rhs switches are the DR-SW
            # bottleneck: ~190ns/MM rotating vs ~85ns/MM constant)
            for jj in range(PAIRS):
                w_ap = wsl(jj)
                nc.tensor.matmul(
                    pm, hi8[:, jj], w_ap,
                    start=(jj == 0),
                    stop=("noresid" in ab and jj == PAIRS - 1),
                    perf_mode=SW,
                )
                if "noresid" not in ab:
                    nc.tensor.matmul(
                        pm, r8[:, jj], w_ap,
                        start=False, stop=(jj == PAIRS - 1), perf_mode=SW,
                    )
            if "noepi" in ab:
                nc.vector.tensor_copy(ob[:, c * CH : (c + 1) * CH], pm)
            else:
                e1t = etp.tile([P, CH], bf16, name="e1t")
                nc.vector.scalar_tensor_tensor(
                    out=e1t, in0=pm, scalar=sclx,
                    in1=alphab[:, c * CH : (c + 1) * CH],
                    op0=OP.mult, op1=OP.mult,
                )
                if "nogpsadd" in ab:
                    nc.vector.tensor_copy(ob[:, c * CH : (c + 1) * CH], e1t)
                else:
                    nc.vector.tensor_tensor(
                        ob[:, c * CH : (c + 1) * CH], e1t,
                        biasb_bf[:, c * CH : (c + 1) * CH], op=OP.add,
                    )

        for rep in range(reps):
            fr = {0: frontend(0)}
            prep_all()
            obs, ndone = {}, {}
            for s, units in enumerate(steps):
                for t, c in units:
                    if t not in obs:
                        obs[t] = outp.tile([P, DOUT], bf16, name="ob", bufs=5)
                        ndone[t] = 0
                    unit(t, c, fr[t], obs[t])
                    ndone[t] += 1
                    if ndone[t] == NCH:
                        # one merged SWDGE store per s-tile
                        nc.gpsimd.dma_start(
                            out=o_d[t * P : (t + 1) * P, :], in_=obs.pop(t)
                        )
                # just-in-time frontends: emit those first used next step
                for t in range(ST):
                    if t not in fr and first_use.get(t) == s + 1:
                        fr[t] = frontend(t)
                for t in list(fr):
                    if last_use[t] <= s:
                        fr.pop(t)

    nc.compile()
    return nc


_CACHE = {}


def _get_nc(g_one=True):
    key = ("nc", g_one)
    if key not in _CACHE:
        _CACHE[key] = build_module(g_one=g_one)
    return _CACHE[key]


def kernel(**inputs) -> np.ndarray:
    g = np.asarray(inputs["g"], dtype=np.float32)
    nc = _get_nc(bool(np.all(g == 1.0)))
    x = np.ascontiguousarray(
        np.asarray(inputs["x"], dtype=np.float32).reshape(B * S, DIN)
    )
    shards = np.split(x, NCORES, axis=0)
    base = {
        k: np.ascontiguousarray(np.asarray(inputs[k], dtype=np.float32))
        for k in ("weight", "row_scale", "bias", "g")
    }
    in_maps = [{"x": shards[c], **base} for c in range(NCORES)]
    res = bass_utils.run_bass_kernel_spmd(nc, in_maps, list(range(NCORES)))
    out = np.concatenate(
        [np.asarray(res.results[c]["out"]) for c in range(NCORES)], axis=0
    )
    return out.reshape(B, S, DOUT).astype(np.float32)


# revision 24
# speedup vs baseline: 1.3283x; 1.0295x over previous
"""BitLinear (input-RMSNorm + ternary-quantized linear) on 8 TRN2 NeuronCores.

Math (reference):
  xn    = x * rsqrt(mean(x^2, -1) + eps) * g
  w     = weight * rsqrt(mean(weight^2, 1) + eps)          (row RMS norm)
  am    = mean(|w|, 1)
  w_q   = sign(w) * (|w| > 0.5*am)                          (ternary)
  out   = xn @ (w_q * am * row_scale).T + bias

Kernel strategy (per core, data-parallel over B*S rows; weight replicated):
  - fp8e4 DoubleRowSwInterleave matmuls (K=256/issue, measured ~94 ns per
    N=512 MM = ~2x the bf16 MAC rate).  The ternary weight {-1,0,1} is
    EXACT in fp8; x rides as fp8(main) + fp8(residual = x - fp8(x)), two
    accumulating GEMM chains into one PSUM bank, so the matmul path loses
    nothing vs bf16 accuracy (~2e-3 rel err vs the 2e-2 gate).
  - alpha = am*rw*row_scale stays f32: broadcast across partitions via a
    tiny PE ones-outer-product per weight tile, applied in the epilogue
    STT (psum*sclx)*alpha on DVE; bias added on gpsimd in bf16.
  - x row rsqrt (sclx) commutes with the matmul (applied per-partition in
    the epilogue STT); g is per-partition in the transposed domain and
    rides the PSUM->SBUF fp8 conversion copies (skipped when g == 1).
  - |w| > 0.5*mean|w| evaluated in the raw-weight domain (rsqrt cancels).
  - x transposed on PE in f32r (exact); main fp8 = ACT copy-cast from
    PSUM, residual = DVE (psum - main) -> fp8, written straight into the
    k-pair-interleaved layouts the DR matmuls consume.
  - Software-pipelined skew: 8 weight tiles prep ahead, 2 per step after;
    (s-tile, chunk) units gate only on the 4 preps their columns need.
  - DMA: x+w loads on the sync HWDGE ring; merged bf16 stores per s-tile
    issue from the gpsimd SWDGE queue.
"""

import sys

try:
    import concourse.bass  # noqa: F401
except ImportError:
    for _p in ("/opt/trn_rl_repo", "/root/.axon_site/_ro/trn_rl_repo"):
        if _p not in sys.path:
            sys.path.insert(0, _p)

from contextlib import ExitStack

import numpy as np

import concourse.bass as bass
import concourse.mybir as mybir
import concourse.tile as tile
from concourse import bacc, bass_utils
from concourse.masks import make_identity

B, S, DIN, DOUT = 4, 4096, 2048, 2048
NCORES = 8
SC = B * S // NCORES      # 2048 rows of x per core
P = 128
KT = DIN // P             # 16 k-tiles
PAIRS = KT // 2           # 8 k-pairs per DR chain
ST = SC // P              # 16 s-tiles per core
CH = 512                  # psum chunk (one bank of fp32)
NCH = DOUT // CH          # 4 chunks
EPS = 1e-8
EHEAD = 8                 # weight tiles prepped before the main loop
RPACE = 2                 # weight tiles prepped per early main-loop step

f32 = mybir.dt.float32
f32r = mybir.dt.float32r
bf16 = mybir.dt.bfloat16
f8 = mybir.dt.float8e4
AF = mybir.ActivationFunctionType
OP = mybir.AluOpType
AX = mybir.AxisListType
SW = mybir.MatmulPerfMode.DoubleRowSwInterleave


def _skew_schedule(ehead=None, rpace=None):
    """Greedy (tile, chunk) unit order: chunk c is eligible once its 4
    preps are done; units process oldest-tile-first, <=4 per step."""
    ehead = EHEAD if ehead is None else ehead
    rpace = RPACE if rpace is None else rpace
    steps = []
    pend = []
    npreps = ehead
    arrived = 0
    for s in range(ST + 4):
        while arrived < ST and arrived <= s + 1:
            pend += [(arrived, c) for c in range(NCH)]
            arrived += 1
        elig = sorted(u for u in pend if NCH * (u[1] + 1) <= npreps and u[0] <= s)
        take = elig[:NCH]
        for u in take:
            pend.remove(u)
        steps.append(take)
        npreps = min(KT, npreps + rpace)
    assert not pend, pend
    return steps


def _fr_lifetimes(steps):
    first_use, last_use = {}, {}
    for s, us in enumerate(steps):
        for t, _ in us:
            first_use.setdefault(t, s)
            last_use[t] = s
    alive = max(
        sum(1 for t in first_use if first_use[t] <= s <= last_use[t])
        for s in range(len(steps))
    )
    return first_use, last_use, alive


def build_module(reps=1, g_one=True, ehead=None, rpace=None, ablate=()):
    ab = set(ablate.split(",")) if isinstance(ablate, str) else set(ablate)
    nc = bacc.Bacc("TRN2", target_bir_lowering=False)
    x_d = nc.declare_dram_parameter("x", [SC, DIN], f32, isOutput=False)
    w_d = nc.declare_dram_parameter("weight", [DOUT, DIN], f32, isOutput=False)
    rs_d = nc.declare_dram_parameter("row_scale", [DOUT, 1], f32, isOutput=False)
    b_d = nc.declare_dram_parameter("bias", [DOUT], f32, isOutput=False)
    g_d = nc.declare_dram_parameter("g", [DIN], f32, isOutput=False)
    o_d = nc.declare_dram_parameter("out", [SC, DOUT], bf16, isOutput=True)

    with tile.TileContext(nc) as tc, ExitStack() as ctx:
        const = ctx.enter_context(tc.tile_pool(name="const", bufs=1))
        xtp = ctx.enter_context(tc.tile_pool(name="xtp", bufs=2))
        wtp = ctx.enter_context(tc.tile_pool(name="wtp", bufs=2))
        abp = ctx.enter_context(tc.tile_pool(name="abp", bufs=2))
        epp = ctx.enter_context(tc.tile_pool(name="epp", bufs=2))
        ehead_v = EHEAD if ehead is None else ehead
        rpace_v = RPACE if rpace is None else rpace
        steps = _skew_schedule(4, 2)
        first_use, last_use, alive = _fr_lifetimes(steps)
        hip = ctx.enter_context(tc.tile_pool(name="hip", bufs=ST + 1))
        rip = ctx.enter_context(tc.tile_pool(name="rip", bufs=ST + 1))
        etp = ctx.enter_context(tc.tile_pool(name="etp", bufs=6))
        smp = ctx.enter_context(tc.tile_pool(name="smp", bufs=4))
        pmm = ctx.enter_context(tc.tile_pool(name="pmm", bufs=3, space="PSUM"))
        ptp = ctx.enter_context(tc.tile_pool(name="ptp", bufs=2, space="PSUM"))
        dram = ctx.enter_context(tc.tile_pool(name="dram", bufs=2, space="DRAM"))

        # ---- constants ----
        w2a = const.tile([P, PAIRS, 2, CH], f8)    # ternary, chunk 0 (local prep)
        w2b = const.tile([P, PAIRS, 2, DOUT - CH], f8)  # chunks 1-3 (AllGather)
        biasb_bf = const.tile([P, DOUT], bf16)
        alphab = const.tile([P, DOUT], f32)        # alpha broadcast to all partitions
        ident32 = const.tile([P, P], f32)
        identbf = const.tile([P, P], bf16)
        make_identity(nc, ident32)
        nc.vector.tensor_copy(identbf, ident32)
        ident32r = ident32.bitcast(f32r)
        eps_t = const.tile([P, 1], f32)
        nc.vector.memset(eps_t, EPS)
        ones1 = const.tile([1, P], f32)
        nc.vector.memset(ones1, 1.0)
        g_col = const.tile([P, KT], f32)           # g[k], k = j*128+p -> [p, j]
        nc.gpsimd.dma_start(out=g_col, in_=g_d.rearrange("(j p) -> p j", p=P))

        # per-w-tile stats: cols 0-1 = AG shard, cols 2-9 = local chunks 0-1
        sabs = const.tile([P, 10], f32)
        rw = const.tile([P, 10], f32)
        traw = const.tile([P, 10], f32)
        ntraw = const.tile([P, 10], f32)
        alpha_sh = const.tile([P, 2], f32)
        alpha_c = const.tile([P, KT], f32)
        w2sh = const.tile([P, PAIRS, 2, 2 * P], f8)
        # (biasb f32 staging uses an xtp slot; only the bf16 copy persists)
        rs_sh = const.tile([P, 10], f32)

        # bias broadcast: DRAM [DOUT] replicated over 128 partitions
        bias_ap = b_d[:]
        biasb_stg = xtp.tile([P, DIN], f32, name="xt")
        nc.gpsimd.dma_start(
            out=biasb_stg,
            in_=bass.AP(
                tensor=bias_ap.tensor, offset=bias_ap.offset,
                ap=[[0, P]] + list(bias_ap.ap),
            ),
        )
        nc.vector.tensor_copy(biasb_bf, biasb_stg)

        # ---- weight prep ----
        # Shard: core c owns o rows [256c, 256c+256) -> ternary fp8 shard,
        # AllGathered for chunks 1-3 (w2b).  Chunk 0 (o < 512) is ALSO
        # prepped locally on every core (w2a) so the GEMM starts while the
        # collective is in flight.  AG readback rides the gpsimd queue so
        # the sync ring stays free for x loads.
        pid = nc.sync.partition_id()

        def prep_core(w_src_ap, rs_src_ap, col, e_sink):
            """stats + ternarize one o-tile; e_sink(e_t) consumes the ternary."""
            w_t = wtp.tile([P, DIN], f32, name="wt")
            nc.sync.dma_start(out=w_t, in_=w_src_ap)
            nc.sync.dma_start(out=rs_sh[:, col : col + 1], in_=rs_src_ap)
            scr4 = smp.tile([P, 4], f32, name="scr4")
            for c in range(4):
                dump = pmm.tile([P, CH], f32, name="dump", bufs=1)
                nc.scalar.activation(
                    dump, w_t[:, c * CH : (c + 1) * CH], AF.Square,
                    accum_out=scr4[:, c : c + 1],
                )
            nc.vector.tensor_tensor(scr4[:, 0:1], scr4[:, 0:1], scr4[:, 1:2], op=OP.add)
            nc.vector.tensor_tensor(scr4[:, 2:3], scr4[:, 2:3], scr4[:, 3:4], op=OP.add)
            nc.vector.tensor_tensor(scr4[:, 0:1], scr4[:, 0:1], scr4[:, 2:3], op=OP.add)
            nc.scalar.activation(
                rw[:, col : col + 1], scr4[:, 0:1], AF.Sqrt, bias=eps_t, scale=1.0 / DIN
            )
            nc.vector.reciprocal(rw[:, col : col + 1], rw[:, col : col + 1])
            nc.vector.tensor_reduce(
                sabs[:, col : col + 1], w_t, axis=AX.X, op=OP.add,
                apply_absolute_value=True,
            )
            nc.vector.tensor_scalar(
                traw[:, col : col + 1], sabs[:, col : col + 1], 0.5 / DIN, None,
                op0=OP.mult,
            )
            nc.vector.tensor_scalar(
                ntraw[:, col : col + 1], sabs[:, col : col + 1], -0.5 / DIN, None,
                op0=OP.mult,
            )
            # alpha = 0.5*mean|w|*rw*rs  (0.5: ternary e is in {-2,0,2})
            nc.vector.tensor_scalar(
                scr4[:, 1:2], sabs[:, col : col + 1], 0.5 / DIN, None, op0=OP.mult
            )
            nc.vector.tensor_tensor(
                scr4[:, 1:2], scr4[:, 1:2], rw[:, col : col + 1], op=OP.mult
            )
            alpha_dst = alpha_sh[:, col : col + 1] if col < 2 else \
                alpha_c[:, col - 2 : col - 1]
            nc.vector.tensor_tensor(
                alpha_dst, scr4[:, 1:2], rs_sh[:, col : col + 1], op=OP.mult
            )
            # ternary sign trick: e = Sign(w-t) + Sign(w+t) in {-2,0,2}
            a_t = abp.tile([P, DIN], bf16, name="at")
            b_t = abp.tile([P, DIN], bf16, name="bt")
            e_t = epp.tile([P, DIN], bf16, name="et")
            nc.scalar.activation(a_t, w_t, AF.Sign, bias=ntraw[:, col : col + 1])
            nc.scalar.activation(b_t, w_t, AF.Sign, bias=traw[:, col : col + 1])
            nc.vector.tensor_tensor(e_t, a_t, b_t, op=OP.add)
            e_sink(e_t)

        def tr_sink(dst_tile, ocol):
            def sink(e_t):
                for kk in range(KT // 4):
                    pt = ptp.tile([P, 2, 2, P], bf16, name="pt", bufs=1)
                    for q in range(4):
                        k = kk * 4 + q
                        nc.tensor.transpose(
                            pt[:, q // 2, q % 2, :],
                            e_t[:, k * P : (k + 1) * P], identbf,
                        )
                    dst = dst_tile[:, 2 * kk : 2 * kk + 2, :,
                                   ocol * P : (ocol + 1) * P]
                    if kk % 2 == 0:
                        nc.scalar.activation(dst, pt, AF.Copy)
                    else:
                        nc.vector.tensor_copy(dst, pt)
            return sink

        def bcast_alpha(j):
            aps = pmm.tile([P, P], f32, name="alps", bufs=1)
            nc.tensor.transpose(aps[0:1, :], alpha_c[:, j : j + 1], ident32)
            art = smp.tile([1, P], f32, name="art", bufs=2)
            nc.vector.tensor_copy(art, aps[0:1, :])
            nc.tensor.matmul(aps, ones1, art, start=True, stop=True)
            nc.vector.tensor_copy(alphab[:, j * P : (j + 1) * P], aps)

        def prep_all():
            # shard prep + AG launch first (collective runs in background)
            for i in range(2):
                prep_core(
                    w_d[bass.ds(pid * (2 * P) + i * P, P), :],
                    rs_d[bass.ds(pid * (2 * P) + i * P, P), :],
                    i, tr_sink(w2sh, i),
                )
            pay = dram.tile([P, PAIRS * 2 * 2 * P + 8], f8, name="pay")
            nc.gpsimd.dma_start(pay[:, : PAIRS * 2 * 2 * P], w2sh[:].opt())
            nc.gpsimd.dma_start(
                pay[:, PAIRS * 2 * 2 * P :], alpha_sh[:].bitcast(f8)
            )
            bout = dram.tile([NCORES * P, PAIRS * 2 * 2 * P + 8], f8, name="bout")
            nc.gpsimd.collective_compute(
                "AllGather",
                OP.bypass,
                replica_groups=[list(range(NCORES))],
                ins=[pay.opt()],
                outs=[bout.opt()],
            )
            # local chunk-0 prep overlaps the AG
            for j in range(4):
                prep_core(
                    w_d[j * P : (j + 1) * P, :],
                    rs_d[j * P : (j + 1) * P, :],
                    2 + j, tr_sink(w2a, j),
                )
                bcast_alpha(j)
            # AG readback (gpsimd queue, FIFO after the collective):
            # chunks 1-3 = shards of cores 2..7
            for c in range(2, NCORES):
                src = bout[c * P : (c + 1) * P, : PAIRS * 2 * 2 * P].rearrange(
                    "p (a b o) -> p a b o", a=PAIRS, b=2
                )
                nc.gpsimd.dma_start(
                    out=w2b[:, :, :, (c - 2) * 2 * P : (c - 1) * 2 * P], in_=src
                )
                nc.gpsimd.dma_start(
                    out=alpha_c[:, 2 * c : 2 * c + 2],
                    in_=bout[
                        c * P : (c + 1) * P, PAIRS * 2 * 2 * P :
                    ].bitcast(f32),
                )
            for j in range(4, KT):
                bcast_alpha(j)

        # ---- frontend: load x tile t, stats, transpose, fp8 main+residual ----
        def frontend(t):
            x_t = xtp.tile([P, DIN], f32, name="xt")
            nc.sync.dma_start(out=x_t, in_=x_d[t * P : (t + 1) * P, :])
            if "nofront" in ab:
                hi8 = hip.tile([P, PAIRS, 2, P], f8, name="hi8")
                r8 = rip.tile([P, PAIRS, 2, P], f8, name="r8")
                sclx = smp.tile([P, 1], f32, name="sclx", bufs=ST + 1)
                nc.vector.memset(hi8, 0.0)
                nc.vector.memset(r8, 0.0)
                nc.vector.tensor_reduce(
                    sclx, x_t[:, :4], axis=AX.X, op=OP.add,
                )
                return hi8, r8, sclx
            scr4 = smp.tile([P, 4], f32, name="scr4")
            for c in range(4):
                dump = pmm.tile([P, CH], f32, name="dump", bufs=1)
                nc.scalar.activation(
                    dump, x_t[:, c * CH : (c + 1) * CH], AF.Square,
                    accum_out=scr4[:, c : c + 1],
                )
            sclx = smp.tile([P, 1], f32, name="sclx", bufs=ST + 1)
            nc.vector.tensor_tensor(
                scr4[:, 0:1], scr4[:, 0:1], scr4[:, 1:2], op=OP.add
            )
            nc.vector.tensor_tensor(
                scr4[:, 2:3], scr4[:, 2:3], scr4[:, 3:4], op=OP.add
            )
            nc.vector.tensor_tensor(
                scr4[:, 0:1], scr4[:, 0:1], scr4[:, 2:3], op=OP.add
            )
            nc.scalar.activation(
                sclx, scr4[:, 0:1], AF.Sqrt, bias=eps_t, scale=1.0 / DIN
            )
            nc.vector.reciprocal(sclx, sclx)

            hi8 = hip.tile([P, PAIRS, 2, P], f8, name="hi8")
            r8 = rip.tile([P, PAIRS, 2, P], f8, name="r8")

            # DoubleRowSwInterleave expects the STATIONARY operand laid out
            # software-interleaved per 256-wide pair-block: flat[254-2m+ko]
            # holds the (k-tile 2jj+ko, out-row m) value.  The transposes
            # write PSUM through strided APs so the blocks are born in that
            # layout and every SBUF conversion below stays contiguous.
            def ilv_pt(pt_, jjh, ko):
                t_ap = pt_[:]
                return bass.AP(
                    tensor=t_ap.tensor,
                    offset=t_ap.offset + jjh * 256 + 254 + ko,
                    ap=[list(t_ap.ap[0]), [-2, P]],
                )

            for kk in range(KT // 4):
                pt = ptp.tile([P, 2, 2, P], f32, name="ptx")
                for q in range(4):
                    k = kk * 4 + q
                    nc.tensor.transpose(
                        ilv_pt(pt, q // 2, q % 2),
                        x_t[:, k * P : (k + 1) * P], ident32,
                    )
                hsl = hi8[:, 2 * kk : 2 * kk + 2, :, :]
                rsl = r8[:, 2 * kk : 2 * kk + 2, :, :]
                if g_one:
                    # main fp8 on ACT, residual (psum - main) on DVE
                    nc.scalar.activation(hsl, pt, AF.Copy)
                    if "noresid" not in ab:
                        nc.vector.tensor_tensor(rsl, pt, hsl, op=OP.subtract)
                else:
                    # general-g path: per-k-tile scale over the strided
                    # interleaved positions (slower; benchmark data has g=1)
                    for q in range(4):
                        k = kk * 4 + q
                        src = ilv_pt(pt, q // 2, q % 2)
                        hq_ap = bass.AP(
                            tensor=hsl.tensor,
                            offset=hsl.offset + (q // 2) * 256 + 254 + (q % 2),
                            ap=[list(hsl.ap[0]), [-2, P]],
                        )
                        rq_ap = bass.AP(
                            tensor=rsl.tensor,
                            offset=rsl.offset + (q // 2) * 256 + 254 + (q % 2),
                            ap=[list(rsl.ap[0]), [-2, P]],
                        )
                        nc.scalar.activation(
                            hq_ap, src, AF.Copy, scale=g_col[:, k : k + 1]
                        )
                        nc.vector.scalar_tensor_tensor(
                            out=rq_ap, in0=src,
                            scalar=g_col[:, k : k + 1], in1=hq_ap,
                            op0=OP.mult, op1=OP.subtract,
                        )
            return hi8, r8, sclx

        # ---- paired units: two s-tiles x one chunk -> rhs-const 4-MM runs ----
        def unit2(frs, t0, t1, c):
            wsl = (
                (lambda jj: w2a[:, jj, :, :])
                if c == 0
                else (lambda jj: w2b[:, jj, :, (c - 1) * CH : c * CH])
            )
            pms_ = []
            for t in (t0, t1):
                pm = pmm.tile([P, CH], f32, name="pm")
                pms_.append((t, pm))
            for jj in range(PAIRS):
                w_ap = wsl(jj)
                for t, pm in pms_:
                    hi8, r8, _ = frs[t]
                    nc.tensor.matmul(
                        pm, hi8[:, jj], w_ap,
                        start=(jj == 0), stop=False, perf_mode=SW,
                    )
                    nc.tensor.matmul(
                        pm, r8[:, jj], w_ap,
                        start=False, stop=(jj == PAIRS - 1), perf_mode=SW,
                    )
            for t, pm in pms_:
                sclx = frs[t][2]
                e1t = etp.tile([P, CH], bf16, name="e1t")
                nc.vector.scalar_tensor_tensor(
                    out=e1t, in0=pm, scalar=sclx,
                    in1=alphab[:, c * CH : (c + 1) * CH],
                    op0=OP.mult, op1=OP.mult,
                )
                obc = etp.tile([P, CH], bf16, name="e2t")
                nc.vector.tensor_tensor(
                    obc, e1t, biasb_bf[:, c * CH : (c + 1) * CH], op=OP.add
                )
                nc.gpsimd.dma_start(
                    out=o_d[t * P : (t + 1) * P, c * CH : (c + 1) * CH],
                    in_=obc,
                )

        for rep in range(reps):
            frs = {}
            frs[0] = frontend(0)
            prep_all()
            # chunk-0 pass streams while the AllGather is in flight
            for tp in range(ST // 2):
                t0, t1 = 2 * tp, 2 * tp + 1
                if t0 not in frs:
                    frs[t0] = frontend(t0)
                if t1 not in frs:
                    frs[t1] = frontend(t1)
                unit2(frs, t0, t1, 0)
            for c in (1, 2, 3):
                for tp in range(ST // 2):
                    unit2(frs, 2 * tp, 2 * tp + 1, c)

    nc.compile()
    return nc


_CACHE = {}


def _get_nc(g_one=True):
    key = ("nc", g_one)
    if key not in _CACHE:
        _CACHE[key] = build_module(g_one=g_one)
    return _CACHE[key]


def kernel(**inputs) -> np.ndarray:
    g = np.asarray(inputs["g"], dtype=np.float32)
    nc = _get_nc(bool(np.all(g == 1.0)))
    x = np.ascontiguousarray(
        np.asarray(inputs["x"], dtype=np.float32).reshape(B * S, DIN)
    )
    shards = np.split(x, NCORES, axis=0)
    base = {
        k: np.ascontiguousarray(np.asarray(inputs[k], dtype=np.float32))
        for k in ("weight", "row_scale", "bias", "g")
    }
    in_maps = [{"x": shards[c], **base} for c in range(NCORES)]
    res = bass_utils.run_bass_kernel_spmd(nc, in_maps, list(range(NCORES)))
    out = np.concatenate(
        [np.asarray(res.results[c]["out"]) for c in range(NCORES)], axis=0
    )
    return out.reshape(B, S, DOUT).astype(np.float32)
